# revision 1
# baseline (speedup 1.0000x reference)
"""Trainium2 Bass kernel for AcceleratedAttentionPool1d.

Key algebraic insight: the reference materializes full [B,S,K,K] window
attention but only keeps the CENTER row (k=pad) of each window. So per
output position s we need:
  - Qtok = Wq @ xp + bq  (per-token query projection over padded seq)
  - energy[s, j] = <Qtok[:, s+4], Qtok[:, s0+j]> / 24 over a 9-wide band
  - attn = softmax over the 9 band entries
  - u[:, s] = sum_j attn[s, j] * xp[:, s0+j]   (V = raw window tokens)
  - out[:, s] = (Wo @ u + bo) / 9

Sharding: data-parallel over batch; B=8 batches on 8 cores, one each.

Per-core dataflow (S processed in 18 chunks of C=120 with halo H=128):
  banded energy = one [120,128] PE matmul pair per chunk, written in
  subgroups of SOFT_G chunks into a single PSUM bank so the band mask
  add / exp / row-sum / reciprocal / normalize run as one fused op set
  per subgroup (free-axis reductions only); one PE transpose per chunk
  turns attention into [H,C]. The output projection is folded into the
  V side by associativity: WXh[j,f] = sum_e xp[e,s0+j]*(Wo/9)[f,e] is
  computed per chunk straight from xp (chunk offset lives in the free
  dim, so no x^T input or halo DMAs are needed), and attn @ WXh yields
  the final output directly — no separate AV + fin matmuls. Bias adds
  ride on the ScalarE PSUM->SBUF evictions; /9 and weight transposes
  are folded into host-side prep; output DMAs batch GROUP=4 chunks.

MODE selects matmul precision: 'fp32' (exact, PE double-pass),
'fp32r' (single-pass fp32, ~56-58us, absmax rel err ~2.9e-4, default),
'bf16' (bf16 operands).
"""

import numpy as np
import ml_dtypes

import concourse.bass as bass
import concourse.mybir as mybir
import concourse.tile as tile
from concourse import bacc
from concourse.bass import ts
from concourse.bass_utils import run_bass_kernel_spmd

F32 = mybir.dt.float32
BF16 = mybir.dt.bfloat16
F32R = mybir.dt.float32r

B, E, S = 8, 256, 2048
KERNEL = 9
PAD = KERNEL // 2
SP = S + 2 * PAD  # 2056
SCALE = 1.0 / (np.sqrt(E) * 1.5)  # 1/24
C = 120  # output positions per chunk
H = 128  # halo width (C + KERNEL - 1)
NCHUNK = 18  # 17 full strides + 1 overlapping tail chunk
import os

GROUP = int(os.environ.get("K_GROUP", "4"))  # chunks per output-proj group
SOFT_G = int(os.environ.get("K_SOFTG", "2"))  # chunks per fused softmax
NEG = -1.0e30

MODE = "fp32r"  # 'fp32' | 'fp32r' | 'bf16'

_T_CHUNKS = [(0, 512), (512, 512), (1024, 512), (1536, 512), (2048, 8)]


def _chunk_start(c: int) -> int:
    return 120 * c if c < NCHUNK - 1 else S - C  # last chunk overlaps


def _groups():
    """Yield lists of chunk indices per output-projection group."""
    out = []
    for g0 in range(0, NCHUNK, GROUP):
        out.append(list(range(g0, min(g0 + GROUP, NCHUNK))))
    return out


def build_nc(mode=None) -> bass.Bass:
    mode = mode or MODE
    # matmul-operand storage dtype
    mdt = {"bf16": BF16, "fp32r": F32R, "fp32": F32}[mode]

    def mm_ap(ap):
        return ap

    nc = bacc.Bacc("TRN2", target_bir_lowering=False)

    xp_d = nc.dram_tensor("xp", [E, SP], mdt, kind="ExternalInput")
    wqt_d = nc.dram_tensor("wqt", [E, E], mdt, kind="ExternalInput")
    wot_d = nc.dram_tensor("wot", [E, E], mdt, kind="ExternalInput")
    bq_d = nc.dram_tensor("bqv", [128, 2], F32, kind="ExternalInput")
    bo_d = nc.dram_tensor("bov", [128, 2], F32, kind="ExternalInput")
    mask_d = nc.dram_tensor("mask", [C, H], F32, kind="ExternalInput")
    id_d = nc.dram_tensor("ident", [128, 128], mdt, kind="ExternalInput")
    out_d = nc.dram_tensor("out", [E, S], F32, kind="ExternalOutput")

    with tile.TileContext(nc) as tc:
        with (
            tc.tile_pool(name="const", bufs=1) as const,
            tc.tile_pool(
                name="work", bufs=int(os.environ.get("K_WBUFS", "4"))
            ) as work,
            tc.tile_pool(
                name="grp", bufs=int(os.environ.get("K_GBUFS", "2"))
            ) as grp,
        ):
            # constants first: the first matmul needs wqt, and DMAs drain
            # in issue order on the sync queue
            wqt_t = const.tile([128, 2, E], mdt)
            nc.sync.dma_start(wqt_t, wqt_d[:, :].rearrange("(i p) f -> p i f", p=128))
            bq_t = const.tile([128, 2], F32)
            nc.sync.dma_start(bq_t, bq_d[:, :])
            # NOTE: keep ALL DMAs on the sync ring — routing any intake
            # through the scalar engine's ring measured consistently slower
            # (62.8us vs 54.9us), even when ScalarE is otherwise idle.
            wot_t = const.tile([128, 2, E], mdt)
            nc.sync.dma_start(wot_t, wot_d[:, :].rearrange("(i p) f -> p i f", p=128))
            xp_view = xp_d[:, :].rearrange("(i p) t -> p i t", p=128)
            xp_t = const.tile([128, 2, SP], mdt)
            for t0, w in _T_CHUNKS[:1]:
                for e_i in range(2):
                    nc.sync.dma_start(
                        xp_t[:, e_i, t0 : t0 + w], xp_view[:, e_i, t0 : t0 + w]
                    )
            mask_t = const.tile([C, H], F32)
            nc.sync.dma_start(mask_t, mask_d[:, :])
            id_t = const.tile([128, 128], mdt)
            nc.sync.dma_start(id_t, id_d[:, :])
            bo_t = const.tile([128, 2], F32)
            nc.sync.dma_start(bo_t, bo_d[:, :])
            for t0, w in _T_CHUNKS[1:]:
                for e_i in range(2):
                    nc.sync.dma_start(
                        xp_t[:, e_i, t0 : t0 + w], xp_view[:, e_i, t0 : t0 + w]
                    )

            qtok_t = const.tile([128, 2, SP], mdt)
            # Wo-projected window tokens for every chunk's halo:
            # WXh[c][j, f] = sum_e xp[e, s0+j] * (Wo/9)[f, e].
            # Associativity folds the output projection into the V side, so
            # later attn @ WXh produces fin directly. These depend only on
            # xp + wot, so they are computed here, interleaved with stage 1
            # by xp-piece arrival, to keep PE fed during the DMA intake.
            wxh_t = const.tile([128, NCHUNK, E], mdt)

            # Stage 1: Qtok[f, t] = sum_e Wq[f, e] xp[e, t] + bq[f]
            with (
                tc.tile_pool(name="psq", bufs=2, space="PSUM") as psq,
                tc.tile_pool(name="pswx", bufs=3, space="PSUM") as pswx,
            ):
                done_wx = 0
                for t0, w in _T_CHUNKS:
                    for f_i in range(2):
                        pq = psq.tile([128, 512], F32)
                        for e_i in range(2):
                            nc.tensor.matmul(
                                pq[:, :w],
                                lhsT=mm_ap(wqt_t[:, e_i, ts(f_i, 128)]),
                                rhs=mm_ap(xp_t[:, e_i, t0 : t0 + w]),
                                start=(e_i == 0),
                                stop=(e_i == 1),
                            )
                        nc.scalar.activation(
                            qtok_t[:, f_i, t0 : t0 + w],
                            pq[:, :w],
                            mybir.ActivationFunctionType.Identity,
                            bias=bq_t[:, f_i : f_i + 1],
                            scale=1.0,
                        )
                    # WXh for every chunk whose halo lies in the xp prefix
                    while done_wx < NCHUNK and (
                        _chunk_start(done_wx) + H <= t0 + w
                    ):
                        s0 = _chunk_start(done_wx)
                        pwx = pswx.tile([128, E], F32)
                        for e_i in range(2):
                            nc.tensor.matmul(
                                pwx,
                                lhsT=mm_ap(xp_t[:, e_i, s0 : s0 + H]),
                                rhs=mm_ap(wot_t[:, e_i, :]),
                                start=(e_i == 0),
                                stop=(e_i == 1),
                            )
                        nc.scalar.copy(wxh_t[:, done_wx, :], pwx)
                        done_wx += 1
                assert done_wx == NCHUNK

            out_view = out_d[:, :].rearrange("(i p) s -> p i s", p=128)

            with (
                tc.tile_pool(name="pse", bufs=3, space="PSUM") as pse,
                tc.tile_pool(name="psat", bufs=2, space="PSUM") as psat,
                tc.tile_pool(name="psf", bufs=3, space="PSUM") as psf,
            ):
                for grp_chunks in _groups():
                    ng = len(grp_chunks)
                    gw = ng * C
                    fo = grp.tile([128, 2, gw], F32, tag="fo")

                    for sg0 in range(0, ng, SOFT_G):
                        sg_chunks = grp_chunks[sg0 : sg0 + SOFT_G]
                        sg = len(sg_chunks)
                        # banded energies for the subgroup into ONE psum
                        # bank: [C, sg, H]; each chunk a [C, H] free-slice
                        pe_ = pse.tile([C, SOFT_G, H], F32)
                        for gi, c in enumerate(sg_chunks):
                            s0 = _chunk_start(c)
                            for f_i in range(2):
                                nc.tensor.matmul(
                                    pe_[:, gi, :],
                                    lhsT=mm_ap(
                                        qtok_t[:, f_i, s0 + PAD : s0 + PAD + C]
                                    ),
                                    rhs=mm_ap(qtok_t[:, f_i, s0 : s0 + H]),
                                    start=(f_i == 0),
                                    stop=(f_i == 1),
                                )
                        # fused subgroup softmax
                        nc.vector.tensor_tensor(
                            out=pe_[:, :sg, :],
                            in0=pe_[:, :sg, :],
                            in1=mask_t[:, None, :].to_broadcast((C, sg, H)),
                            op=mybir.AluOpType.add,
                        )
                        A = work.tile([128, SOFT_G, H], mdt, tag="A")
                        nc.gpsimd.memset(
                            A[96:128, :, :].bitcast(mybir.dt.uint32), 0
                        )
                        nc.scalar.activation(
                            A[:C, :sg, :],
                            pe_[:, :sg, :],
                            mybir.ActivationFunctionType.Exp,
                            scale=SCALE,
                        )
                        sums = work.tile([C, SOFT_G], F32, tag="sums")
                        nc.vector.tensor_reduce(
                            sums[:, :sg],
                            A[:C, :sg, :],
                            axis=mybir.AxisListType.X,
                            op=mybir.AluOpType.add,
                        )
                        r = work.tile([C, SOFT_G], F32, tag="r")
                        nc.vector.reciprocal(r[:, :sg], sums[:, :sg])
                        nc.vector.tensor_tensor(
                            out=A[:C, :sg, :],
                            in0=A[:C, :sg, :],
                            in1=r[:, :sg, None].to_broadcast((C, sg, H)),
                            op=mybir.AluOpType.mult,
                        )
                        # transpose attention tiles to [H, C]; both subgroup
                        # chunks share one PSUM tile and one eviction
                        pat = psat.tile([128, SOFT_G, 128], mdt)
                        for gi in range(sg):
                            nc.tensor.transpose(pat[:, gi, :], A[:, gi, :], id_t)
                        at = work.tile([128, SOFT_G, 128], mdt, tag="at")
                        if os.environ.get("K_ATACT", "0") == "1":
                            nc.scalar.copy(at[:, :sg, :], pat[:, :sg, :])
                        else:
                            nc.vector.tensor_copy(at[:, :sg, :], pat[:, :sg, :])
                        for gi, c in enumerate(sg_chunks):
                            # fin[f, s] = sum_j WXh[j, f] * at[j, s]  (+bias)
                            pf = psf.tile([128, 2, C], F32)
                            for f_i in range(2):
                                nc.tensor.matmul(
                                    pf[:, f_i, :],
                                    lhsT=mm_ap(wxh_t[:, c, ts(f_i, 128)]),
                                    rhs=mm_ap(at[:, gi, :C]),
                                    start=True,
                                    stop=True,
                                )
                            g0 = (sg0 + gi) * C
                            for f_i in range(2):
                                nc.scalar.activation(
                                    fo[:, f_i, g0 : g0 + C],
                                    pf[:, f_i, :],
                                    mybir.ActivationFunctionType.Identity,
                                    bias=bo_t[:, f_i : f_i + 1],
                                    scale=1.0,
                                )
                    # store: non-tail chunks in a group are contiguous in S,
                    # so they go out as one DMA; the overlapping tail chunk
                    # contributes only its last 8 columns
                    plain = [c for c in grp_chunks if c < NCHUNK - 1]
                    if plain:
                        s0 = _chunk_start(plain[0])
                        nc.sync.dma_start(
                            out_view[:, :, s0 : s0 + len(plain) * C],
                            fo[:, :, : len(plain) * C],
                        )
                    if grp_chunks[-1] == NCHUNK - 1:
                        gi = len(grp_chunks) - 1
                        s0 = _chunk_start(NCHUNK - 1)
                        d0 = 120 * (NCHUNK - 1) - s0  # 112
                        nc.sync.dma_start(
                            out_view[:, :, s0 + d0 : s0 + C],
                            fo[:, :, gi * C + d0 : (gi + 1) * C],
                        )
    nc.compile()
    return nc


def make_in_maps(x, Wq, bq, Wo, bo, mode=None):
    mode = mode or MODE
    npdt = ml_dtypes.bfloat16 if mode == "bf16" else np.float32

    x = np.asarray(x, dtype=np.float32)
    Wq = np.asarray(Wq, dtype=np.float32)
    bq = np.asarray(bq, dtype=np.float32)
    Wo = np.asarray(Wo, dtype=np.float32)
    bo = np.asarray(bo, dtype=np.float32)

    wqt = np.ascontiguousarray(Wq.T).astype(npdt)
    wot = np.ascontiguousarray((Wo / KERNEL).T).astype(npdt)
    bqv = np.ascontiguousarray(bq.reshape(2, 128).T)
    bov = np.ascontiguousarray((bo / KERNEL).reshape(2, 128).T)

    mask = np.full((C, H), NEG, dtype=np.float32)
    for m in range(C):
        mask[m, m : m + KERNEL] = 0.0
    ident = np.eye(128, dtype=npdt)

    in_maps = []
    for b in range(B):
        xp = np.zeros((E, SP), dtype=np.float32)
        xp[:, PAD : PAD + S] = x[b]
        in_maps.append(
            dict(
                xp=xp.astype(npdt),
                wqt=wqt,
                wot=wot,
                bqv=bqv,
                bov=bov,
                mask=mask,
                ident=ident,
            )
        )
    return in_maps


_NC_CACHE = {}


def kernel(x, Wq, bq, Wo, bo):
    res = kernel_with_results(x, Wq, bq, Wo, bo)
    return np.stack([r["out"] for r in res.results]).astype(np.float32)


def kernel_with_results(x, Wq, bq, Wo, bo, trace=False, mode=None, **kwargs):
    in_maps = make_in_maps(x, Wq, bq, Wo, bo, mode=mode)
    key = mode or MODE
    if key not in _NC_CACHE:
        _NC_CACHE[key] = build_nc(mode=mode)
    return run_bass_kernel_spmd(
        _NC_CACHE[key], in_maps, core_ids=list(range(B)), trace=trace, **kwargs
    )



# revision 7
# speedup vs baseline: 1.3211x; 1.3211x over previous
"""Trainium2 Bass kernel for AcceleratedAttentionPool1d (v2).

Algebra: only the CENTER row of each window's attention survives, so per
output position s:
  qtok = (Wq @ xp + bq)/sqrt(24)            (scale folded into weights)
  energy[s, j] = <qtok[:, s+4], qtok[:, s0+j]>  over a 9-wide band
  attn = softmax(energy) over the band
  out[:, s] = (Wo/9) @ (sum_j attn[s,j] xp[:, s0+j]) + bo/9
The output projection folds into the V side: WXh[c][j, f] = sum_e
xp[e, s0+j]*(Wo/9)[f, e], so attn @ WXh is the final output directly.

Sharding: data-parallel over batch; B=8 batches on 8 cores.

v2 structure (vs v1):
 - bf16 matmul operands AND bf16 output (host converts to fp32):
   fp32r matmuls with free-dim <256 run at 4 cyc/row on the PE; bf16 is
   1 cyc/row everywhere. Measured rel err ~4e-3 vs the 2e-2 gate.
 - DMA: 9 input dma_starts (was 33), each a packed [128, bytes]
   partition-contiguous blob (128 descriptors), split across the sync/
   scalar/gpsimd rings so issue (~5ns/descriptor, serial per ring)
   overlaps. Output is a packed [128, 2, S] bf16 dram tensor the host
   unpacks; 4 output dma_starts on the idle sync ring.
 - Stage 1 (qtok+WXh) and the attention phase are INTERLEAVED by xp
   prefix arrival, so PE stays continuously busy (p-state: the PE runs
   2x slower unless busy; gaps reset it).
 - Softmax in [C, H] orientation, SOFT_G=3 chunks fused per PSUM bank.
 - Engine balance: qtok evict=vector(tensor_scalar_add bias), wxh
   evict=gpsimd, exp=scalar, reduce=gpsimd, recip+norm+mask=vector,
   at evict=gpsimd, fin evict=scalar(bias); A tiles persistent with
   one-time pad-row memset (no per-subgroup memsets).
 - PSUM: psq 2 + pswx 1 + pse 2 + psat 1 + psf 2 = 8 banks.
"""

import os
import numpy as np
import ml_dtypes

import concourse.bass as bass
import concourse.mybir as mybir
import concourse.tile as tile
from concourse import bacc
from concourse.bass import ts
from concourse.bass_utils import run_bass_kernel_spmd

F32 = mybir.dt.float32
BF16 = mybir.dt.bfloat16

B, E, S = 8, 256, 2048
KERNEL = 9
PAD = KERNEL // 2
SP = S + 2 * PAD  # 2056
C = 120  # output positions per chunk
H = 128  # halo width
NCHUNK = 18  # 17 full strides + 1 overlapping tail chunk
SOFT_G = 3  # chunks per fused softmax subgroup
NSG = NCHUNK // SOFT_G
GROUP = 6  # chunks per output tile/store
NEG = -1.0e30
TA, TB = 1024, SP - 1024  # xp dma split

T_CH = [(0, 512), (512, 512), (1024, 512), (1536, 512), (2048, 8)]


def _cs(c: int) -> int:
    return 120 * c if c < NCHUNK - 1 else S - C  # last chunk overlaps


def build_nc() -> bass.Bass:
    nc = bacc.Bacc("TRN2", target_bir_lowering=False)

    x0a_d = nc.dram_tensor("x0a", [128, TA], BF16, kind="ExternalInput")
    x0b_d = nc.dram_tensor("x0b", [128, TB], BF16, kind="ExternalInput")
    x1a_d = nc.dram_tensor("x1a", [128, TA], BF16, kind="ExternalInput")
    x1b_d = nc.dram_tensor("x1b", [128, TB], BF16, kind="ExternalInput")
    wqt_d = nc.dram_tensor("wqt", [128, 2, E], BF16, kind="ExternalInput")
    wot_d = nc.dram_tensor("wot", [128, 2, E], BF16, kind="ExternalInput")
    mask_d = nc.dram_tensor("maskd", [128, H], F32, kind="ExternalInput")
    id_d = nc.dram_tensor("identd", [128, 128], BF16, kind="ExternalInput")
    bq_d = nc.dram_tensor("bqvd", [128, 2], F32, kind="ExternalInput")
    bo_d = nc.dram_tensor("bovd", [128, 2], F32, kind="ExternalInput")
    out_d = nc.dram_tensor("out", [128, 2, S], BF16, kind="ExternalOutput")

    with tile.TileContext(nc) as tc:
        with (
            tc.tile_pool(name="const", bufs=1) as const,
            tc.tile_pool(name="work", bufs=4) as work,
            tc.tile_pool(name="grp", bufs=2) as grp,
            tc.tile_pool(name="ps", bufs=1, space="PSUM") as ps,
        ):
            # ---- DMA intake: sync ring carries weights+x (in consumption
            # order); scalar/gpsimd rings carry the small consts in parallel.
            wqt_t = const.tile([128, 2, E], BF16)
            nc.sync.dma_start(wqt_t, wqt_d[:, :, :])
            x0_t = const.tile([128, SP], BF16)
            x1_t = const.tile([128, SP], BF16)
            nc.sync.dma_start(x0_t[:, 0:TA], x0a_d[:, :])
            nc.sync.dma_start(x1_t[:, 0:TA], x1a_d[:, :])
            nc.sync.dma_start(x0_t[:, TA:SP], x0b_d[:, :])
            nc.sync.dma_start(x1_t[:, TA:SP], x1b_d[:, :])

            bq_t = const.tile([128, 2], F32)
            nc.scalar.dma_start(bq_t, bq_d[:, :])
            wot_t = const.tile([128, 2, E], BF16)
            nc.scalar.dma_start(wot_t, wot_d[:, :, :])
            bo_t = const.tile([128, 2], F32)
            nc.scalar.dma_start(bo_t, bo_d[:, :])

            mask_t = const.tile([128, H], F32)
            nc.gpsimd.dma_start(mask_t, mask_d[:, :])
            id_t = const.tile([128, 128], BF16)
            nc.gpsimd.dma_start(id_t, id_d[:, :])

            xs = [x0_t, x1_t]
            qtok0 = const.tile([128, SP], BF16)
            qtok1 = const.tile([128, SP], BF16)
            qtoks = [qtok0, qtok1]
            wxh_t = const.tile([128, NCHUNK, E], BF16)

            # persistent attention tiles; pad rows zeroed once
            A0 = const.tile([128, SOFT_G, H], BF16)
            A1 = const.tile([128, SOFT_G, H], BF16)
            nc.gpsimd.memset(A0[96:128, :, :].bitcast(mybir.dt.uint32), 0)
            nc.gpsimd.memset(A1[96:128, :, :].bitcast(mybir.dt.uint32), 0)
            A_slots = [A0, A1]

            out_view = out_d[:, :, :]
            fo_tiles = {}
            pf_state = {}
            state = {"wx": 0, "sg": 0, "pwx": None}

            def emit_wxh_upto(prefix):
                while state["wx"] < NCHUNK and _cs(state["wx"]) + H <= prefix:
                    c = state["wx"]
                    ci = c % 2
                    if ci == 0:
                        state["pwx"] = ps.tile(
                            [128, 2, E], F32, tag="pswx", bufs=1, name="pwx"
                        )
                    pwx = state["pwx"]
                    s0 = _cs(c)
                    for e_i in range(2):
                        nc.tensor.matmul(
                            pwx[:, ci, :],
                            lhsT=xs[e_i][:, s0 : s0 + H],
                            rhs=wot_t[:, e_i, :],
                            start=(e_i == 0),
                            stop=(e_i == 1),
                        )
                    if ci == 1:
                        nc.scalar.copy(wxh_t[:, c - 1 : c + 1, :], pwx)
                    state["wx"] += 1

            def emit_fin(c, at_ap):
                g = c // GROUP
                p = (c % GROUP) // 2
                ci = c % 2
                if ci == 0:
                    pf_state[(g, p)] = ps.tile(
                        [128, 2, 2, C], F32, tag="psf", bufs=2, name="pf"
                    )
                pf = pf_state[(g, p)]
                for f_i in range(2):
                    nc.tensor.matmul(
                        pf[:, f_i, ci, :],
                        lhsT=wxh_t[:, c, ts(f_i, 128)],
                        rhs=at_ap[:, 0:C],
                        start=True,
                        stop=True,
                    )
                if ci == 1:
                    fo = fo_tiles[g]
                    for f_i in range(2):
                        nc.scalar.activation(
                            fo[:, f_i, 240 * p : 240 * p + 240],
                            pf[:, f_i, :, :],
                            mybir.ActivationFunctionType.Identity,
                            bias=bo_t[:, f_i : f_i + 1],
                            scale=1.0,
                        )

            def emit_sg(k):
                g = k // 2
                if k % 2 == 0:
                    fo_tiles[g] = grp.tile(
                        [128, 2, GROUP * C], BF16, tag="fo", name="fo"
                    )
                pe_ = ps.tile([C, SOFT_G, H], F32, tag="pse", bufs=2, name="pe_")
                for gi in range(SOFT_G):
                    c = SOFT_G * k + gi
                    s0 = _cs(c)
                    for f_i in range(2):
                        nc.tensor.matmul(
                            pe_[:, gi, :],
                            lhsT=qtoks[f_i][:, s0 + PAD : s0 + PAD + C],
                            rhs=qtoks[f_i][:, s0 : s0 + H],
                            start=(f_i == 0),
                            stop=(f_i == 1),
                        )
                nc.vector.tensor_tensor(
                    out=pe_,
                    in0=pe_,
                    in1=mask_t[:C, None, :].to_broadcast((C, SOFT_G, H)),
                    op=mybir.AluOpType.add,
                )
                A = A_slots[k % 2]
                nc.scalar.activation(
                    A[:C, :, :], pe_, mybir.ActivationFunctionType.Exp
                )
                sums = work.tile([C, SOFT_G], F32, tag="sums", name="sums")
                nc.vector.tensor_reduce(
                    sums,
                    A[:C, :, :],
                    axis=mybir.AxisListType.X,
                    op=mybir.AluOpType.add,
                )
                r = work.tile([C, SOFT_G], F32, tag="r", name="r")
                nc.vector.reciprocal(r, sums)
                nc.vector.tensor_tensor(
                    out=A[:C, :, :],
                    in0=A[:C, :, :],
                    in1=r[:, :, None].to_broadcast((C, SOFT_G, H)),
                    op=mybir.AluOpType.mult,
                )
                pat = ps.tile([128, SOFT_G, 128], BF16, tag="psat", bufs=1, name="pat")
                for gi in range(SOFT_G):
                    nc.tensor.transpose(pat[:, gi, :], A[:, gi, :], id_t)
                at = work.tile([128, SOFT_G, 128], BF16, tag="at", name="at")
                nc.vector.tensor_copy(at, pat)
                for gi in range(SOFT_G):
                    emit_fin(SOFT_G * k + gi, at[:, gi, :])
                if k % 2 == 1:
                    fo = fo_tiles[g]
                    s0 = GROUP * C * g
                    if g < 2:
                        nc.sync.dma_start(
                            out_view[:, :, s0 : s0 + GROUP * C], fo
                        )
                    else:
                        nc.sync.dma_start(out_view[:, :, 1440:2040], fo[:, :, 0:600])
                        nc.sync.dma_start(
                            out_view[:, :, 2040:2048], fo[:, :, 712:720]
                        )

            for t0, w in T_CH:
                for f_i in range(2):
                    pq = ps.tile([128, 512], F32, tag="psq", bufs=2, name="pq")
                    for e_i in range(2):
                        nc.tensor.matmul(
                            pq[:, :w],
                            lhsT=wqt_t[:, e_i, ts(f_i, 128)],
                            rhs=xs[e_i][:, t0 : t0 + w],
                            start=(e_i == 0),
                            stop=(e_i == 1),
                        )
                    nc.vector.tensor_scalar_add(
                        qtoks[f_i][:, t0 : t0 + w], pq[:, :w], bq_t[:, f_i : f_i + 1]
                    )
                prefix = t0 + w
                emit_wxh_upto(prefix)
                while (
                    state["sg"] < NSG
                    and _cs(SOFT_G * state["sg"] + 2) + H <= prefix
                ):
                    emit_sg(state["sg"])
                    state["sg"] += 1
            while state["sg"] < NSG:
                emit_sg(state["sg"])
                state["sg"] += 1

    nc.compile()
    return nc


def make_in_maps(x, Wq, bq, Wo, bo):
    x = np.asarray(x, dtype=np.float32)
    Wq = np.asarray(Wq, dtype=np.float32)
    bq = np.asarray(bq, dtype=np.float32)
    Wo = np.asarray(Wo, dtype=np.float32)
    bo = np.asarray(bo, dtype=np.float32)

    bf = ml_dtypes.bfloat16
    f = 1.0 / np.sqrt(np.sqrt(E) * 1.5)  # 1/sqrt(24) folded into Wq, bq
    wqt = np.ascontiguousarray(
        (Wq * f).T.reshape(2, 128, E).transpose(1, 0, 2)
    ).astype(bf)
    wot = np.ascontiguousarray(
        (Wo / KERNEL).T.reshape(2, 128, E).transpose(1, 0, 2)
    ).astype(bf)
    bqv = np.ascontiguousarray((bq * f).reshape(2, 128).T)
    bov = np.ascontiguousarray((bo / KERNEL).reshape(2, 128).T)

    mask = np.full((128, H), NEG, dtype=np.float32)
    for m in range(128):
        mask[m, m : min(m + KERNEL, H)] = 0.0
    ident = np.eye(128, dtype=bf)

    in_maps = []
    for b in range(B):
        xp = np.zeros((E, SP), dtype=np.float32)
        xp[:, PAD : PAD + S] = x[b]
        xpb = xp.astype(bf)
        in_maps.append(
            dict(
                x0a=np.ascontiguousarray(xpb[0:128, 0:TA]),
                x0b=np.ascontiguousarray(xpb[0:128, TA:SP]),
                x1a=np.ascontiguousarray(xpb[128:256, 0:TA]),
                x1b=np.ascontiguousarray(xpb[128:256, TA:SP]),
                wqt=wqt,
                wot=wot,
                maskd=mask,
                identd=ident,
                bqvd=bqv,
                bovd=bov,
            )
        )
    return in_maps


_NC_CACHE = {}


def kernel(x, Wq, bq, Wo, bo):
    res = kernel_with_results(x, Wq, bq, Wo, bo)
    outs = []
    for r in res.results:
        o = np.asarray(r["out"])  # [128, 2, S] bf16
        outs.append(o.transpose(1, 0, 2).reshape(E, S).astype(np.float32))
    return np.stack(outs)


def kernel_with_results(x, Wq, bq, Wo, bo, trace=False, **kwargs):
    in_maps = make_in_maps(x, Wq, bq, Wo, bo)
    if "nc" not in _NC_CACHE:
        _NC_CACHE["nc"] = build_nc()
    return run_bass_kernel_spmd(
        _NC_CACHE["nc"], in_maps, core_ids=list(range(B)), trace=trace, **kwargs
    )
